# revision 23
# baseline (speedup 1.0000x reference)
"""Trainium2 Bass kernel for a 2-layer LSTM + dense + softmax-CE loss.

Model (from the reference):
  B, T, V, E, H = 4096, 80, 80, 8, 256
  x  = emb[features]                  # [B, T, E]
  h1 = LSTM(x;  W1, b1)               # TF BasicLSTMCell, gates (i, j, f, o)
  h2 = LSTM(h1; W2, b2)
  pred = h2[:, -1] @ Wd + bd          # [B, V]
  loss = mean(softmax_xent(pred, labels))

Sharding: pure data parallelism - batch 4096 split 512/core across 8 cores,
weights replicated. Per-core device kernel computes the 512 per-row losses;
host averages the 4096 rows.

v2 design (vs bf16 baseline at ~1.14 ms):
- fp8e4 DoubleRow matmuls for the recurrent weights: K=256 per pass, so
  L1 takes 1 pass and L2 takes 2 passes per 128-row gate tile (24 passes/step
  instead of 48 bf16 ones).
- The tiny K=8 x-projection matmuls (which cost 320 ns each as 8 full
  N=512 passes) are packed 4x with tile_position row-tiling, and the layer-1
  bias rides as a 9th input row of x' = [x; 1] (host folds b1+forget bias
  into the weight).  Layer-2 bias is a K=1 ones-matmul, also 4x packed.
- All gate biases are therefore inside PSUM already, so the activation ops
  need no per-partition bias operand and can merge: sigmoid(i) [2 banks],
  tanh(j) [2 banks], sigmoid(f,o) [one 4-bank op].  ACT work per step drops
  from 20x(512+352) to (1024+1024+2048+2x1024 + overheads) cycles.
- DVE runs in fp16 (2x mode): c kept in fp16, gate outputs fp16, h in fp8
  ready for the next DoubleRow pass.
"""

from contextlib import ExitStack

import numpy as np

B, T, V, E, H = 4096, 80, 80, 8, 256
FORGET_BIAS = 1.0
NCORES = 8
BL = B // NCORES          # 512 batch rows per core
NB = BL // 128            # 4 batch tiles of 128 for the loss stage

_CACHE = {}


def _build_nc(T_steps=T):
    import concourse.tile as tile
    from concourse import bacc, mybir

    f32 = mybir.dt.float32
    bf16 = mybir.dt.bfloat16
    fp16 = mybir.dt.float16
    f8 = mybir.dt.float8e4
    AF = mybir.ActivationFunctionType
    OP = mybir.AluOpType
    DR = mybir.MatmulPerfMode.DoubleRow

    nc = bacc.Bacc("TRN2", target_bir_lowering=False, debug=False)

    XT = nc.dram_tensor("XT", [T, E + 1, BL], bf16, kind="ExternalInput")
    W1XP = nc.dram_tensor("W1XP", [128, 2, 128], bf16, kind="ExternalInput")
    W1HDR = nc.dram_tensor("W1HDR", [128, 2, 8 * 128], f8, kind="ExternalInput")
    W2DRA = nc.dram_tensor("W2DRA", [128, 2, 8 * 128], f8, kind="ExternalInput")
    W2DRB = nc.dram_tensor("W2DRB", [128, 2, 8 * 128], f8, kind="ExternalInput")
    B2P = nc.dram_tensor("B2P", [128, 2, 256], bf16, kind="ExternalInput")
    WD8 = nc.dram_tensor("WD8", [128, 2, V], f8, kind="ExternalInput")
    BD = nc.dram_tensor("BD", [1, V], bf16, kind="ExternalInput")
    OH = nc.dram_tensor("OH", [BL, V], f32, kind="ExternalInput")
    LOSS = nc.dram_tensor("LOSS", [NB, 128], f32, kind="ExternalOutput")

    with tile.TileContext(nc) as tc, ExitStack() as ctx:
        wp = ctx.enter_context(tc.tile_pool(name="weights", bufs=1))
        sp = ctx.enter_context(tc.tile_pool(name="state", bufs=1))
        hp = ctx.enter_context(tc.tile_pool(name="h", bufs=4))
        gp = ctx.enter_context(tc.tile_pool(name="gates", bufs=4))
        xp = ctx.enter_context(tc.tile_pool(name="xstream", bufs=4))
        pp = ctx.enter_context(tc.tile_pool(name="psum", bufs=4, space="PSUM"))
        lp = ctx.enter_context(tc.tile_pool(name="loss", bufs=1))

        # ---- static loads, ordered by first use
        xt0 = xp.tile([128, BL], bf16, tag="xt", name="xt0")
        for g in range(4):
            nc.sync.dma_start(xt0[32 * g : 32 * g + E + 1, :], XT[0])
        w1xp = wp.tile([128, 2, 128], bf16, tag="w1xp")
        nc.sync.dma_start(w1xp[:], W1XP[:])
        b2p = wp.tile([128, 2, 256], bf16, tag="b2p")
        nc.sync.dma_start(b2p[:], B2P[:])
        w2dra = wp.tile([128, 2, 8 * 128], f8, tag="w2dra")
        nc.sync.dma_start(w2dra[:], W2DRA[:])
        w1hdr = wp.tile([128, 2, 8 * 128], f8, tag="w1hdr")
        nc.sync.dma_start(w1hdr[:], W1HDR[:])
        w2drb = wp.tile([128, 2, 8 * 128], f8, tag="w2drb")
        nc.sync.dma_start(w2drb[:], W2DRB[:])
        ones_f = wp.tile([128, BL], f32, tag="ones_f")
        nc.vector.memset(ones_f[:], 1.0)
        ones = wp.tile([128, BL], bf16, tag="ones")
        nc.vector.tensor_copy(ones[:], ones_f[:])
        wd8 = wp.tile([128, 2, V], f8, tag="wd8")
        nc.sync.dma_start(wd8[:], WD8[:])
        bdt = wp.tile([1, V], bf16, tag="bdt")
        nc.sync.dma_start(bdt[:], BD[:])
        oh_tiles = []
        for m in range(NB):
            t_ = lp.tile([128, V], f32, tag=f"oh{m}", name=f"oh{m}")
            nc.sync.dma_start(t_[:], OH[128 * m : 128 * (m + 1), :])
            oh_tiles.append(t_)

        # persistent cell states, fp16, [hidden-half0 | half1] in dim1
        c1 = sp.tile([128, 2, BL], fp16, tag="c1", name="c1")
        c2 = sp.tile([128, 2, BL], fp16, tag="c2", name="c2")

        def small_pass(ps4, w, xt_or_ones, t, krows, start, stop):
            """One 4x row-tiled pass per PSUM half: K<=32 matmuls for the
            x-projection (krows=9) or the L2 bias (krows=1).
            ps4 = (ps_i, ps_j, ps_f, ps_o); pass p group g covers gate tile
            m = 4p + g -> tile ps4[m//2] slot m%2."""
            for p in range(2):
                for g in range(4):
                    m = 4 * p + g
                    r = slice(32 * g, 32 * g + krows)
                    nc.tensor.matmul(
                        ps4[m // 2][:, m % 2, :], w[r, p, 0:128], xt_or_ones[r, :],
                        start=start, stop=stop, tile_position=(32 * g, 0),
                    )

        def dr_pass(ps4, w, h, start, stop):
            for m in range(8):
                nc.tensor.matmul(
                    ps4[m // 2][:, m % 2, :], w[:, :, 128 * m : 128 * (m + 1)],
                    h[:, :, :], start=start, stop=stop, perf_mode=DR,
                )

        def act_dve(t, layer, ps4, c):
            """gates + cell update + h for one layer-step.
            ps4 = per-gate 2-bank PSUM tiles (i, j, f, o)."""
            psi, psj, psf, pso = ps4
            gi = gp.tile([128, 2, BL], fp16, tag=f"gi{layer}")
            nc.scalar.activation(gi[:], psi[:], AF.Sigmoid)
            gj = gp.tile([128, 2, BL], fp16, tag=f"gj{layer}")
            nc.scalar.activation(gj[:], psj[:], AF.Tanh)
            gf = gp.tile([128, 2, BL], fp16, tag=f"gf{layer}")
            nc.scalar.activation(gf[:], psf[:], AF.Sigmoid)
            go = gp.tile([128, 2, BL], fp16, tag=f"go{layer}")
            nc.scalar.activation(go[:], pso[:], AF.Sigmoid)
            if t == 0:
                nc.vector.tensor_tensor(c[:], gi[:], gj[:], op=OP.mult)
            else:
                tmp = gp.tile([128, 2, BL], fp16, tag=f"tmp{layer}")
                nc.vector.tensor_tensor(tmp[:], gi[:], gj[:], op=OP.mult)
                nc.vector.tensor_tensor(c[:], c[:], gf[:], op=OP.mult)
                nc.vector.tensor_tensor(c[:], c[:], tmp[:], op=OP.add)
            hc = gp.tile([128, 2, BL], fp16, tag=f"hc{layer}")
            nc.scalar.activation(hc[:], c[:], AF.Tanh)
            hn = hp.tile([128, 2, BL], f8, tag=f"h{layer}")
            nc.vector.tensor_tensor(hn[:], hc[:], go[:], op=OP.mult)
            return hn

        h1 = h2 = None
        for t in range(T_steps):
            if t == 0:
                xt = xt0
            else:
                xt = xp.tile([128, BL], bf16, tag="xt", name="xt")
                for g in range(4):
                    nc.sync.dma_start(xt[32 * g : 32 * g + E + 1, :], XT[t])
            ps1 = tuple(
                pp.tile([128, 2, BL], f32, tag="ps", name=f"ps1{g}_{t}")
                for g in "ijfo"
            )
            small_pass(ps1, w1xp, xt, t, E + 1, start=True, stop=(t == 0))
            if t > 0:
                dr_pass(ps1, w1hdr, h1, start=False, stop=True)
            h1 = act_dve(t, 1, ps1, c1)

            ps2 = tuple(
                pp.tile([128, 2, BL], f32, tag="ps", name=f"ps2{g}_{t}")
                for g in "ijfo"
            )
            if t > 0:
                dr_pass(ps2, w2drb, h2, start=True, stop=False)
            small_pass(ps2, b2p, ones, t, 1, start=(t == 0), stop=False)
            dr_pass(ps2, w2dra, h1, start=False, stop=True)
            h2 = act_dve(t, 2, ps2, c2)

        # ---- dense + softmax cross-entropy on the last h2 ----
        pds, nmxs, ses, lses, pkss = [], [], [], [], []
        for m in range(NB):
            ms = slice(128 * m, 128 * (m + 1))
            pd = pp.tile([128, V], f32, tag="ps", name=f"pd{m}")
            nc.tensor.matmul(pd[:], h2[:, 0, ms], wd8[:, 0, :], start=True, stop=False)
            nc.tensor.matmul(pd[:], h2[:, 1, ms], wd8[:, 1, :], start=False, stop=False)
            nc.tensor.matmul(pd[:], ones[0:1, ms], bdt[:], start=False, stop=True)
            pds.append(pd)
            mx = lp.tile([128, 1], f32, tag=f"mx{m}")
            nc.vector.reduce_max(out=mx[:], in_=pd[:], axis=mybir.AxisListType.X)
            nmx = lp.tile([128, 1], f32, tag=f"nmx{m}")
            nc.vector.tensor_scalar_mul(nmx[:], mx[:], -1.0)
            nmxs.append(nmx)
        for m in range(NB):
            ex = lp.tile([128, V], f32, tag=f"ex{m}")
            se = lp.tile([128, 1], f32, tag=f"se{m}")
            nc.scalar.activation(ex[:], pds[m][:], AF.Exp, bias=nmxs[m][:], accum_out=se[:])
            ses.append(se)
        for m in range(NB):
            lse = lp.tile([128, 1], f32, tag=f"lse{m}")
            nc.scalar.activation(lse[:], ses[m][:], AF.Ln)
            lses.append(lse)
            pk = lp.tile([128, V], f32, tag=f"pk{m}")
            nc.vector.tensor_tensor(pk[:], pds[m][:], oh_tiles[m][:], op=OP.mult)
            pks = lp.tile([128, 1], f32, tag=f"pks{m}")
            nc.vector.reduce_sum(out=pks[:], in_=pk[:], axis=mybir.AxisListType.X)
            pkss.append(pks)
        for m in range(NB):
            # loss = max + lse - picked  (lse is ln(sum exp(pred - max)))
            l0 = lp.tile([128, 1], f32, tag=f"l0{m}")
            nc.vector.tensor_tensor(l0[:], lses[m][:], pkss[m][:], op=OP.subtract)
            l1_ = lp.tile([128, 1], f32, tag=f"l1{m}")
            nc.vector.tensor_tensor(l1_[:], l0[:], nmxs[m][:], op=OP.subtract)
            nc.sync.dma_start(LOSS[m, :], l1_[:, 0:1])

    nc.compile()
    return nc


def _prep_inputs(features, labels, emb, W1, b1, W2, b2, Wd, bd):
    """Host-side shard + layout prep. Returns in_maps for the 8 cores."""
    import ml_dtypes

    bf16 = ml_dtypes.bfloat16
    f8 = ml_dtypes.float8_e4m3
    features = np.asarray(features)
    labels = np.asarray(labels)
    emb = np.asarray(emb, dtype=np.float32)
    W1 = np.asarray(W1, dtype=np.float32)
    W2 = np.asarray(W2, dtype=np.float32)
    Wd = np.asarray(Wd, dtype=np.float32)

    b1f = np.asarray(b1, dtype=np.float32).copy()
    b1f[2 * H : 3 * H] += FORGET_BIAS
    b2f = np.asarray(b2, dtype=np.float32).copy()
    b2f[2 * H : 3 * H] += FORGET_BIAS

    # x-projection weights + L1 bias as a 9th input row, replicated to the
    # 4 row-tile groups: group g of pass p computes gate tile m = 4p + g.
    W1XP = np.zeros((128, 2, 128), np.float32)
    B2P = np.zeros((128, 2, 256), np.float32)
    for g in range(4):
        for p in range(2):
            m = 4 * p + g
            W1XP[32 * g : 32 * g + E, p, :] = W1[0:E, 128 * m : 128 * (m + 1)]
            W1XP[32 * g + E, p, :] = b1f[128 * m : 128 * (m + 1)]
            B2P[32 * g, p, 0:128] = b2f[128 * m : 128 * (m + 1)]
    W1XP = np.ascontiguousarray(W1XP.astype(bf16))
    B2P = np.ascontiguousarray(B2P.astype(bf16))

    # DoubleRow weight tiles [k, group, m]: group i covers hidden rows
    # 128i..128i+127 of the recurrent input.
    def dr_pack(Wrows):  # [256, 1024] -> [128, 2, 1024]
        return np.ascontiguousarray(
            Wrows.reshape(2, 128, 4 * H).transpose(1, 0, 2).astype(f8)
        )

    W1HDR = dr_pack(W1[E : E + H])
    W2DRA = dr_pack(W2[0:H])
    W2DRB = dr_pack(W2[H : 2 * H])
    WD8 = np.ascontiguousarray(
        Wd.reshape(2, 128, V).transpose(1, 0, 2).astype(f8)
    )
    BDt = np.ascontiguousarray(np.asarray(bd, np.float32).reshape(1, V).astype(bf16))

    x = emb[features]  # [B, T, E] f32
    eye = np.eye(V, dtype=np.float32)

    in_maps = []
    for c in range(NCORES):
        sl = slice(c * BL, (c + 1) * BL)
        xc = x[sl].transpose(1, 2, 0)  # [T, E, BL]
        xp_ = np.empty((T, E + 1, BL), np.float32)
        xp_[:, 0:E, :] = xc
        xp_[:, E, :] = 1.0
        oh = eye[labels[sl]]
        in_maps.append({
            "XT": np.ascontiguousarray(xp_.astype(bf16)),
            "OH": np.ascontiguousarray(oh),
            "W1XP": W1XP, "W1HDR": W1HDR,
            "W2DRA": W2DRA, "W2DRB": W2DRB, "B2P": B2P,
            "WD8": WD8, "BD": BDt,
        })
    return in_maps


def _run(inputs, trace=False, **spmd_kwargs):
    from concourse.bass_utils import run_bass_kernel_spmd

    if "nc" not in _CACHE:
        _CACHE["nc"] = _build_nc()
    nc = _CACHE["nc"]
    in_maps = _prep_inputs(**inputs)
    res = run_bass_kernel_spmd(
        nc, in_maps, list(range(NCORES)), trace=trace, **spmd_kwargs
    )
    rows = np.concatenate([np.asarray(r["LOSS"], np.float64).ravel() for r in res.results])
    loss = np.asarray(rows.mean(), dtype=np.float32)
    return loss, res


def kernel(**inputs):
    loss, _ = _run(inputs, trace=False)
    return loss
